# revision 6
# baseline (speedup 1.0000x reference)
"""Multi-head causal self-attention (B=2, S=2048, D=1024, H=16) on 8 TRN2 cores.

Sharding: core c handles batch b = c//4 and head group g = c%4 (4 heads,
256 output dims). W_q/W_k/W_v are split column-wise per head group, W_o
row-wise; each core computes a partial [S, D] output product which the host
sums per batch (plus the (bv @ Wo.T + bo) row, exact because softmax rows
sum to 1).

Device kernel per core, fully interleaved per 512-query chunk so the PE,
ACT and DVE engines all stay busy across the whole timeline (a separate
projection phase would leave ACT idle for its first ~40us and the exp
stream would then dominate the attention phase):
  per chunk c:
    QT/KT[dl, s-chunk] = w.T @ xT (+ bias)   PE full-array, DVE bias-add
    V[s-chunk, dl|1]   = xT.T @ wv            (ones col -> row sums ride PV)
    scoresT[sk, sq] = KT_h.T @ QT_h   2 heads at partitions 0-63/64-127,
        64x128 PE row tiling -> the pair runs concurrently
    PT = exp(scoresT) (ACT) * causal mask (GPSIMD)
    po_h[dv|sum, sq] += [V_h|1].T @ PT        N=512-x0 streams
  epilogue (deferred one chunk so it hides under the next attention loop):
    rr fp16 = cast(po sums row)  -> row 0 of a zeroed [128,...] tile
    psb = onesr.T @ rr            full-array broadcast (rows 1-127 zero)
    bcr = 1/psb                   DVE reciprocal, OTn = po * bcr (fp16)
    out[s, :] = OTn.T-slice @ woT (fp16 partial, summed on host)
"""

import os
import sys

import numpy as np

# concourse (Bass/Tile) normally comes from PYTHONPATH; fall back to the
# container's copy when run from a bare directory.
for _p in ("/root/.axon_site/_ro/trn_rl_repo", "/opt/trn_rl_repo"):
    if _p not in sys.path and os.path.isdir(_p):
        sys.path.append(_p)

S = 2048
D = 1024
HL = 4          # heads per core
DL = 256        # local head dims per core
SC = 512        # sq chunk width
NSC = S // SC   # 4 chunks
NKT = S // 128  # 16 sk tiles
KC = D // 128   # 8 contraction chunks for the projections

# Matmul operand dtype: fp16 streams 1 col/cycle on the PE (fp32r: 2, fp32: 4)
# and halves the x/w DMA. fp16 is safe here: max exp(score) ~ 490 << 65504,
# verified rel err ~7e-4 end to end.
MM_DTYPE = os.environ.get("BASS_MM_DTYPE", "f16")
TRACE = os.environ.get("BASS_KERNEL_TRACE", "0") == "1"

_CACHE = {}


def _build():
    import concourse.bass as bass
    import concourse.mybir as mybir
    import concourse.tile as tile
    from concourse import bacc

    dt = mybir.dt
    f32 = dt.float32
    mmdt = {"f16": dt.float16, "f32r": dt.float32r, "f32": dt.float32}[MM_DTYPE]

    nc = bacc.Bacc("TRN2", target_bir_lowering=False, debug=False)

    xqT = nc.dram_tensor("xqT", [D, S], mmdt, kind="ExternalInput").ap()
    xkT = nc.dram_tensor("xkT", [D, S], mmdt, kind="ExternalInput").ap()
    xvT = nc.dram_tensor("xvT", [D, S], mmdt, kind="ExternalInput").ap()
    wqT = nc.dram_tensor("wqT", [D, DL], mmdt, kind="ExternalInput").ap()
    wkT = nc.dram_tensor("wkT", [D, DL], mmdt, kind="ExternalInput").ap()
    wvT = nc.dram_tensor("wvT", [D, DL], mmdt, kind="ExternalInput").ap()
    woT = nc.dram_tensor("woT", [DL, D], mmdt, kind="ExternalInput").ap()
    bqd = nc.dram_tensor("bqd", [128, 2], f32, kind="ExternalInput").ap()
    bkd = nc.dram_tensor("bkd", [128, 2], f32, kind="ExternalInput").ap()
    maskd = nc.dram_tensor("maskd", [128, 4, SC], mmdt, kind="ExternalInput").ap()
    outd = nc.dram_tensor("out", [S, D], mmdt, kind="ExternalOutput").ap()

    Exp = mybir.ActivationFunctionType.Exp

    def mm(ps, lhsT, rhs, start, stop):
        nc.tensor.matmul(ps, lhsT, rhs, start=start, stop=stop)

    with tile.TileContext(nc) as tc:
        with (
            tc.tile_pool(name="const", bufs=1) as constp,
            tc.tile_pool(name="wq", bufs=1) as wqp,
            tc.tile_pool(name="wk", bufs=1) as wkp,
            tc.tile_pool(name="wv", bufs=1) as wvp,
            tc.tile_pool(name="wo", bufs=1) as wop,
            tc.tile_pool(name="x", bufs=6) as xp,
            tc.tile_pool(name="qt", bufs=2) as qtp,
            tc.tile_pool(name="kt", bufs=2) as ktp,
            tc.tile_pool(name="v", bufs=NKT) as vp,
            tc.tile_pool(name="pt", bufs=4) as ptp,
            tc.tile_pool(name="otn", bufs=2) as otp,
            tc.tile_pool(name="otr", bufs=2) as orp,
            tc.tile_pool(name="bcr", bufs=2) as bcp,
            tc.tile_pool(name="zr", bufs=1) as zrp,
            tc.tile_pool(name="osb", bufs=3) as osp,
            tc.tile_pool(name="ps", bufs=2, space="PSUM") as psp,
            tc.tile_pool(name="po", bufs=4, space="PSUM") as pop,
        ):
            ones_f32 = constp.tile([128, 64], f32, tag="ones_f32")
            nc.vector.memset(ones_f32[:], 1.0)
            # broadcast weights: row 0 = ones, rows 1-127 = 0, so the
            # broadcast matmul runs in full 128x128 mode (no PE mode switch)
            onesr = constp.tile([128, 128], mmdt, tag="onesr")
            nc.vector.memset(onesr[:], 0.0)
            nc.vector.memset(onesr[0:1, :], 1.0)
            bq_sb = constp.tile([128, 2], f32, tag="bq")
            nc.sync.dma_start(bq_sb[:], bqd[:])
            bk_sb = constp.tile([128, 2], f32, tag="bk")
            nc.sync.dma_start(bk_sb[:], bkd[:])
            mask_sb = constp.tile([128, 4, SC], mmdt, tag="mask")
            nc.sync.dma_start(mask_sb[:], maskd[:])
            # zero-padded broadcast source: row 0 carries the fp16 row sums
            zr = zrp.tile([128, HL, 512], mmdt, tag="zr")
            nc.vector.memset(zr[:], 0.0)

            wq_sb = wqp.tile([128, KC, DL], mmdt, tag="wq")
            nc.sync.dma_start(wq_sb[:], wqT.rearrange("(kc p) n -> p kc n", p=128))
            wk_sb = wkp.tile([128, KC, DL], mmdt, tag="wk")
            nc.sync.dma_start(wk_sb[:], wkT.rearrange("(kc p) n -> p kc n", p=128))
            wv_sb = wvp.tile([128, KC, DL], mmdt, tag="wv")
            nc.sync.dma_start(wv_sb[:], wvT.rearrange("(kc p) n -> p kc n", p=128))
            wo_sb = wop.tile([128, 2, D], mmdt, tag="wo")
            nc.sync.dma_start(wo_sb[:], woT.rearrange("(kc p) n -> p kc n", p=128))

            QT = [qtp.tile([128, S], mmdt, tag="qt", name=f"qt{i}") for i in range(2)]
            KT = [ktp.tile([128, S], mmdt, tag="kt", name=f"kt{i}") for i in range(2)]
            OTn = [otp.tile([128, S], mmdt, tag="otn", name=f"otn{i}") for i in range(2)]
            Vt = [vp.tile([128, HL * 65], mmdt, tag="v", name=f"v{i}") for i in range(NKT)]

            xqr = xqT.rearrange("(kc p) s -> p kc s", p=128)
            xkr = xkT.rearrange("(kc p) s -> p kc s", p=128)
            xvr = xvT.rearrange("(kc p) s -> p kc s", p=128)

            def proj_chunk(sc):
                ssl = slice(sc * SC, (sc + 1) * SC)
                # QT / KT
                for xr, w_sb, dstT, b_sb in (
                    (xqr, wq_sb, QT, bq_sb),
                    (xkr, wk_sb, KT, bk_sb),
                ):
                    xt = xp.tile([128, KC, SC], mmdt, tag="x")
                    nc.sync.dma_start(xt[:], xr[:, :, ssl])
                    ps = psp.tile([128, 1024], f32, tag="ps")
                    for t in range(2):
                        for kc in range(KC):
                            mm(
                                ps[:, t * 512 : (t + 1) * 512],
                                w_sb[:, kc, t * 128 : (t + 1) * 128],
                                xt[:, kc, :],
                                start=(kc == 0),
                                stop=(kc == KC - 1),
                            )
                    for t in range(2):
                        nc.vector.tensor_add(
                            dstT[t][:, ssl],
                            ps[:, t * 512 : (t + 1) * 512],
                            b_sb[:, t : t + 1].broadcast_to([128, SC]),
                        )
                # V
                xt = xp.tile([128, KC, SC], mmdt, tag="x")
                nc.sync.dma_start(xt[:], xvr[:, :, ssl])
                for pair in range(2):
                    ps = psp.tile([128, 1024], f32, tag="ps")
                    for sub in range(2):
                        st = sc * 4 + pair * 2 + sub
                        off = sub * 512
                        for kc in range(KC):
                            mm(
                                ps[:, off : off + DL],
                                xt[:, kc, (pair * 2 + sub) * 128 : (pair * 2 + sub + 1) * 128],
                                wv_sb[:, kc, :],
                                start=(kc == 0),
                                stop=(kc == KC - 1),
                            )
                        dst = Vt[st].rearrange("p (h x) -> p h x", x=65)
                        nc.vector.tensor_copy(
                            dst[:, :, 0:64],
                            ps[:, off : off + DL].rearrange("p (h x) -> p h x", x=64),
                        )
                        nc.vector.tensor_copy(
                            dst[:, :, 64:65],
                            ones_f32[:, None, 0:1].broadcast_to([128, HL, 1]),
                        )

            def attn_block(c, j, po):
                d = j - 4 * c  # >= 0 on the block diagonal
                x0 = max(0, 128 * d)  # first non-fully-masked sq column
                jmax = 4 * c + 3
                for pr in range(2):
                    ps = psp.tile([128, 1024], f32, tag="ps")
                    for h2 in range(2):
                        h = pr * 2 + h2
                        t, p0 = divmod(h, 2)
                        psl = slice(p0 * 64, p0 * 64 + 64)
                        mm(
                            ps[:, h2 * 512 + x0 : (h2 + 1) * 512],
                            KT[t][psl, j * 128 : (j + 1) * 128],
                            QT[t][psl, c * SC + x0 : (c + 1) * SC],
                            start=True,
                            stop=True,
                        )
                    pt = ptp.tile([128, 1024], mmdt, tag="pt")
                    psv = ps.rearrange("p (h x) -> p h x", x=512)
                    ptv = pt.rearrange("p (h x) -> p h x", x=512)
                    nc.scalar.activation(ptv[:, :, x0:], psv[:, :, x0:], Exp)
                    if d >= 0:
                        # triangular mask on the 128-wide diagonal block
                        nc.gpsimd.tensor_mul(
                            ptv[:, :, x0 : x0 + 128],
                            ptv[:, :, x0 : x0 + 128],
                            mask_sb[:, 0:1, 0:128].broadcast_to([128, 2, 128]),
                        )
                    for h2 in range(2):
                        h = pr * 2 + h2
                        mm(
                            po[h][:, x0:],
                            Vt[j][:, 65 * h : 65 * h + 65],
                            pt[:, h2 * 512 + x0 : (h2 + 1) * 512],
                            start=(j == 0),
                            stop=(j == jmax),
                        )

            def epilogue(c, otr, csl):
                # fp16 row sums into row 0 of the zero-padded broadcast src
                nc.vector.tensor_copy(zr[0:1, :, :], otr[64:65, :, :])
                for t in range(2):
                    psb = psp.tile([128, 1024], f32, tag="ps", name=f"psb{c}_{t}")
                    for p0 in range(2):
                        h = 2 * t + p0
                        mm(
                            psb[:, p0 * 512 : (p0 + 1) * 512],
                            onesr[:],
                            zr[:, h, :],
                            start=True,
                            stop=True,
                        )
                    bcr = bcp.tile([64, 2, 512], f32, tag="bcr", name=f"bcr{c}_{t}")
                    nc.vector.reciprocal(
                        bcr[:], psb[0:64, :].rearrange("p (a x) -> p a x", x=512)
                    )
                    for p0 in range(2):
                        h = 2 * t + p0
                        nc.vector.tensor_mul(
                            OTn[t][p0 * 64 : p0 * 64 + 64, csl],
                            otr[0:64, h, :],
                            bcr[:, p0, :],
                        )
                for st in range(4 * c, 4 * c + 4):
                    pso = psp.tile([128, 1024], f32, tag="ps")
                    for n in range(2):
                        for k2 in range(2):
                            mm(
                                pso[:, n * 512 : (n + 1) * 512],
                                OTn[k2][:, st * 128 : (st + 1) * 128],
                                wo_sb[:, k2, n * 512 : (n + 1) * 512],
                                start=(k2 == 0),
                                stop=(k2 == 1),
                            )
                    osb = osp.tile([128, D], mmdt, tag="osb")
                    if st % 2 == 0:
                        nc.vector.tensor_copy(osb[:], pso[:])
                    else:
                        nc.scalar.copy(osb[:], pso[:])
                    nc.sync.dma_start(outd[st * 128 : (st + 1) * 128, :], osb[:])

            pending = None
            for c in range(NSC):
                csl = slice(c * SC, (c + 1) * SC)
                proj_chunk(c)
                po = [
                    pop.tile([65, 512], f32, tag="po", name=f"po{c}_{h}")
                    for h in range(HL)
                ]
                for j in range(4 * c + 4):
                    attn_block(c, j, po)
                # po is copied to SBUF right away so its PSUM banks free for
                # the next chunk. The normalization + out-projection for this
                # chunk is emitted AFTER the next chunk's projections and
                # attention (epilogue()), so the PE never head-of-line blocks
                # on the DVE normalize chain.
                otr = orp.tile([65, HL, 512], f32, tag="otr", name=f"otr{c}")
                for h in range(HL):
                    nc.vector.tensor_copy(otr[:, h, :], po[h][:])
                if pending is not None:
                    pending()
                pending = (lambda c=c, otr=otr, csl=csl: epilogue(c, otr, csl))
            pending()

    nc.compile()
    return nc


def _get_nc():
    key = ("nc", MM_DTYPE)
    if key not in _CACHE:
        _CACHE[key] = _build()
    return _CACHE[key]


def make_in_maps(q, k, v, Wq, bq, Wk, bk, Wv, bv, Wo, bo):
    """Host-side shard prep: per-core input dict."""
    f32 = np.float32
    md = {"f16": np.float16, "f32r": f32, "f32": f32}[MM_DTYPE]
    masks = (
        np.arange(SC)[None, None, :]
        >= (128 * np.arange(4)[None, :, None] + np.arange(128)[:, None, None])
    ).astype(md)
    # per-batch transposes shared by the 4 cores of each batch
    xqT = [np.ascontiguousarray(q[b].T.astype(md)) for b in range(2)]
    xkT = [np.ascontiguousarray(k[b].T.astype(md)) for b in range(2)]
    xvT = [np.ascontiguousarray(v[b].T.astype(md)) for b in range(2)]
    in_maps = []
    for c in range(8):
        b, g = c // 4, c % 4
        sl = slice(DL * g, DL * (g + 1))
        in_maps.append(
            {
                "xqT": xqT[b],
                "xkT": xkT[b],
                "xvT": xvT[b],
                "wqT": np.ascontiguousarray(((Wq[sl, :].T) * f32(0.125)).astype(md)),
                "wkT": np.ascontiguousarray(Wk[sl, :].T.astype(md)),
                "wvT": np.ascontiguousarray(Wv[sl, :].T.astype(md)),
                "woT": np.ascontiguousarray(Wo[:, sl].T.astype(md)),
                "bqd": np.ascontiguousarray((bq[sl] * f32(0.125)).reshape(2, 128).T),
                "bkd": np.ascontiguousarray(bk[sl].reshape(2, 128).T),
                "maskd": masks,
            }
        )
    return in_maps


def kernel(q, k, v, Wq, bq, Wk, bk, Wv, bv, Wo, bo):
    from concourse.bass_utils import run_bass_kernel_spmd

    args = [np.asarray(a, dtype=np.float32) for a in (q, k, v, Wq, bq, Wk, bk, Wv, bv, Wo, bo)]
    q, k, v, Wq, bq, Wk, bk, Wv, bv, Wo, bo = args
    nc = _get_nc()
    in_maps = make_in_maps(q, k, v, Wq, bq, Wk, bk, Wv, bv, Wo, bo)
    tmpdir = os.environ.get("BASS_KERNEL_TMPDIR") or None
    res = run_bass_kernel_spmd(nc, in_maps, list(range(8)), trace=TRACE, tmpdir=tmpdir)
    if TRACE and res.exec_time_ns is not None:
        print(f"HW exec time: {res.exec_time_ns} ns")
        print(f"HW exec time mean: {res.mean_exec_time_ns} ns")
    out = np.zeros((2, S, D), np.float32)
    for c in range(8):
        out[c // 4] += res.results[c]["out"]
    out += (bv @ Wo.T + bo)[None, None, :]
    return out


# revision 12
# speedup vs baseline: 1.1403x; 1.1403x over previous
"""Multi-head causal self-attention (B=2, S=2048, D=1024, H=16) on 8 TRN2 cores.

Sharding: core c handles batch b = c//4 and head group g = c%4 (4 heads,
256 output dims). W_q/W_k/W_v are split column-wise per head group, W_o
row-wise; each core computes a partial [S, D] output product which the host
sums per batch (plus the (bv @ Wo.T + bo) row, exact because softmax rows
sum to 1).

Device kernel per core, fully interleaved per 512-query chunk so the PE,
ACT and DVE engines all stay busy across the whole timeline (a separate
projection phase would leave ACT idle for its first ~40us and the exp
stream would then dominate the attention phase):
  per chunk c:
    QT/KT[dl, s-chunk] = w.T @ xT (+ bias)   PE full-array, DVE bias-add
    V[s-chunk, dl|1]   = xT.T @ wv            (ones col -> row sums ride PV)
    scoresT[sk, sq] = KT_h.T @ QT_h   2 heads at partitions 0-63/64-127,
        64x128 PE row tiling -> the pair runs concurrently
    PT = exp(scoresT) (ACT) * causal mask (GPSIMD)
    po_h[dv|sum, sq] += [V_h|1].T @ PT        N=512-x0 streams
  epilogue (deferred one chunk so it hides under the next attention loop):
    rr fp16 = cast(po sums row)  -> row 0 of a zeroed [128,...] tile
    psb = onesr.T @ rr            full-array broadcast (rows 1-127 zero)
    bcr = 1/psb                   DVE reciprocal, OTn = po * bcr (fp16)
    out[s, :] = OTn.T-slice @ woT (fp16 partial, summed on host)
"""

import os
import sys

import numpy as np

# concourse (Bass/Tile) normally comes from PYTHONPATH; fall back to the
# container's copy when run from a bare directory.
for _p in ("/root/.axon_site/_ro/trn_rl_repo", "/opt/trn_rl_repo"):
    if _p not in sys.path and os.path.isdir(_p):
        sys.path.append(_p)

S = 2048
D = 1024
HL = 4          # heads per core
DL = 256        # local head dims per core
SC = 512        # sq chunk width
NSC = S // SC   # 4 chunks
NKT = S // 128  # 16 sk tiles
KC = D // 128   # 8 contraction chunks for the projections

# Matmul operand dtype: fp16 streams 1 col/cycle on the PE (fp32r: 2, fp32: 4)
# and halves the x/w DMA. fp16 is safe here: max exp(score) ~ 490 << 65504,
# verified rel err ~7e-4 end to end.
MM_DTYPE = os.environ.get("BASS_MM_DTYPE", "f16")
TRACE = os.environ.get("BASS_KERNEL_TRACE", "0") == "1"

_CACHE = {}


def _build():
    import concourse.bass as bass
    import concourse.mybir as mybir
    import concourse.tile as tile
    from concourse import bacc

    dt = mybir.dt
    f32 = dt.float32
    mmdt = {"f16": dt.float16, "f32r": dt.float32r, "f32": dt.float32}[MM_DTYPE]

    nc = bacc.Bacc("TRN2", target_bir_lowering=False, debug=False)

    xqT = nc.dram_tensor("xqT", [D, S], mmdt, kind="ExternalInput").ap()
    xkT = nc.dram_tensor("xkT", [D, S], mmdt, kind="ExternalInput").ap()
    xvT = nc.dram_tensor("xvT", [D, S], mmdt, kind="ExternalInput").ap()
    wqT = nc.dram_tensor("wqT", [D, DL], mmdt, kind="ExternalInput").ap()
    wkT = nc.dram_tensor("wkT", [D, DL], mmdt, kind="ExternalInput").ap()
    wvT = nc.dram_tensor("wvT", [D, DL], mmdt, kind="ExternalInput").ap()
    woT = nc.dram_tensor("woT", [DL, D], mmdt, kind="ExternalInput").ap()
    bqd = nc.dram_tensor("bqd", [128, 2], f32, kind="ExternalInput").ap()
    bkd = nc.dram_tensor("bkd", [128, 2], f32, kind="ExternalInput").ap()
    maskd = nc.dram_tensor("maskd", [128, 4, SC], mmdt, kind="ExternalInput").ap()
    outd = nc.dram_tensor("out", [S, D], mmdt, kind="ExternalOutput").ap()

    Exp = mybir.ActivationFunctionType.Exp

    def mm(ps, lhsT, rhs, start, stop):
        nc.tensor.matmul(ps, lhsT, rhs, start=start, stop=stop)

    with tile.TileContext(nc) as tc:
        with (
            tc.tile_pool(name="const", bufs=1) as constp,
            tc.tile_pool(name="wq", bufs=1) as wqp,
            tc.tile_pool(name="wk", bufs=1) as wkp,
            tc.tile_pool(name="wv", bufs=1) as wvp,
            tc.tile_pool(name="wo", bufs=1) as wop,
            tc.tile_pool(name="x", bufs=6) as xp,
            tc.tile_pool(name="qt", bufs=2) as qtp,
            tc.tile_pool(name="kt", bufs=2) as ktp,
            tc.tile_pool(name="v", bufs=NKT) as vp,
            tc.tile_pool(name="pt", bufs=4) as ptp,
            tc.tile_pool(name="otn", bufs=2) as otp,
            tc.tile_pool(name="otr", bufs=2) as orp,
            tc.tile_pool(name="bcr", bufs=2) as bcp,
            tc.tile_pool(name="zr", bufs=1) as zrp,
            tc.tile_pool(name="osb", bufs=3) as osp,
            tc.tile_pool(name="ps", bufs=2, space="PSUM") as psp,
            tc.tile_pool(name="po", bufs=4, space="PSUM") as pop,
        ):
            ones_f32 = constp.tile([128, 64], f32, tag="ones_f32")
            nc.vector.memset(ones_f32[:], 1.0)
            # broadcast weights: row 0 = ones, rows 1-127 = 0, so the
            # broadcast matmul runs in full 128x128 mode (no PE mode switch)
            onesr = constp.tile([128, 128], mmdt, tag="onesr")
            nc.vector.memset(onesr[:], 0.0)
            nc.vector.memset(onesr[0:1, :], 1.0)
            # zero-padded broadcast source: row 0 carries the fp16 row sums
            zr = zrp.tile([128, HL, 512], mmdt, tag="zr")
            nc.vector.memset(zr[:], 0.0)
            bq_sb = constp.tile([128, 2], f32, tag="bq")
            nc.sync.dma_start(bq_sb[:], bqd[:])
            bk_sb = constp.tile([128, 2], f32, tag="bk")
            nc.sync.dma_start(bk_sb[:], bkd[:])
            mask_sb = constp.tile([128, 4, SC], mmdt, tag="mask")
            nc.sync.dma_start(mask_sb[:], maskd[:])

            wq_sb = wqp.tile([128, KC, DL], mmdt, tag="wq")
            nc.sync.dma_start(wq_sb[:], wqT.rearrange("(kc p) n -> p kc n", p=128))
            wk_sb = wkp.tile([128, KC, DL], mmdt, tag="wk")
            nc.sync.dma_start(wk_sb[:], wkT.rearrange("(kc p) n -> p kc n", p=128))
            wv_sb = wvp.tile([128, KC, DL], mmdt, tag="wv")
            nc.sync.dma_start(wv_sb[:], wvT.rearrange("(kc p) n -> p kc n", p=128))
            wo_sb = wop.tile([128, 2, D], mmdt, tag="wo")
            nc.sync.dma_start(wo_sb[:], woT.rearrange("(kc p) n -> p kc n", p=128))

            QT = [qtp.tile([128, S], mmdt, tag="qt", name=f"qt{i}") for i in range(2)]
            KT = [ktp.tile([128, S], mmdt, tag="kt", name=f"kt{i}") for i in range(2)]
            OTn = [otp.tile([128, S], mmdt, tag="otn", name=f"otn{i}") for i in range(2)]
            Vt = [vp.tile([128, HL * 65], mmdt, tag="v", name=f"v{i}") for i in range(NKT)]

            xqr = xqT.rearrange("(kc p) s -> p kc s", p=128)
            xkr = xkT.rearrange("(kc p) s -> p kc s", p=128)
            xvr = xvT.rearrange("(kc p) s -> p kc s", p=128)

            def proj_chunk(sc):
                ssl = slice(sc * SC, (sc + 1) * SC)
                # QT / KT
                for xr, w_sb, dstT, b_sb in (
                    (xqr, wq_sb, QT, bq_sb),
                    (xkr, wk_sb, KT, bk_sb),
                ):
                    xt = xp.tile([128, KC, SC], mmdt, tag="x")
                    nc.sync.dma_start(xt[:], xr[:, :, ssl])
                    ps = psp.tile([128, 1024], f32, tag="ps")
                    for t in range(2):
                        for kc in range(KC):
                            mm(
                                ps[:, t * 512 : (t + 1) * 512],
                                w_sb[:, kc, t * 128 : (t + 1) * 128],
                                xt[:, kc, :],
                                start=(kc == 0),
                                stop=(kc == KC - 1),
                            )
                    for t in range(2):
                        nc.vector.tensor_add(
                            dstT[t][:, ssl],
                            ps[:, t * 512 : (t + 1) * 512],
                            b_sb[:, t : t + 1].broadcast_to([128, SC]),
                        )
                # V
                xt = xp.tile([128, KC, SC], mmdt, tag="x")
                nc.sync.dma_start(xt[:], xvr[:, :, ssl])
                for pair in range(2):
                    ps = psp.tile([128, 1024], f32, tag="ps")
                    for sub in range(2):
                        st = sc * 4 + pair * 2 + sub
                        off = sub * 512
                        for kc in range(KC):
                            mm(
                                ps[:, off : off + DL],
                                xt[:, kc, (pair * 2 + sub) * 128 : (pair * 2 + sub + 1) * 128],
                                wv_sb[:, kc, :],
                                start=(kc == 0),
                                stop=(kc == KC - 1),
                            )
                        dst = Vt[st].rearrange("p (h x) -> p h x", x=65)
                        nc.vector.tensor_copy(
                            dst[:, :, 0:64],
                            ps[:, off : off + DL].rearrange("p (h x) -> p h x", x=64),
                        )
                        nc.vector.tensor_copy(
                            dst[:, :, 64:65],
                            ones_f32[:, None, 0:1].broadcast_to([128, HL, 1]),
                        )

            def attn_block(c, j, po):
                d = j - 4 * c  # >= 0 on the block diagonal
                x0 = max(0, 128 * d)  # first non-fully-masked sq column
                jmax = 4 * c + 3
                for pr in range(2):
                    ps = psp.tile([128, 1024], f32, tag="ps")
                    for h2 in range(2):
                        h = pr * 2 + h2
                        t, p0 = divmod(h, 2)
                        psl = slice(p0 * 64, p0 * 64 + 64)
                        mm(
                            ps[:, h2 * 512 + x0 : (h2 + 1) * 512],
                            KT[t][psl, j * 128 : (j + 1) * 128],
                            QT[t][psl, c * SC + x0 : (c + 1) * SC],
                            start=True,
                            stop=True,
                        )
                    pt = ptp.tile([128, 1024], mmdt, tag="pt")
                    psv = ps.rearrange("p (h x) -> p h x", x=512)
                    ptv = pt.rearrange("p (h x) -> p h x", x=512)
                    nc.scalar.activation(ptv[:, :, x0:], psv[:, :, x0:], Exp)
                    if d >= 0:
                        # triangular mask on the 128-wide diagonal block
                        nc.gpsimd.tensor_mul(
                            ptv[:, :, x0 : x0 + 128],
                            ptv[:, :, x0 : x0 + 128],
                            mask_sb[:, 0:1, 0:128].broadcast_to([128, 2, 128]),
                        )
                    for h2 in range(2):
                        h = pr * 2 + h2
                        mm(
                            po[h][:, x0:],
                            Vt[j][:, 65 * h : 65 * h + 65],
                            pt[:, h2 * 512 + x0 : (h2 + 1) * 512],
                            start=(j == 0),
                            stop=(j == jmax),
                        )

            def normalize(c, otr, csl):
                # Broadcast the fp16 row sums (planted in row 0 of the zeroed
                # zr tile) across 64 partitions with a full-array matmul,
                # then 1/x with the fast-approx custom DVE op (18-bit, plenty
                # for a softmax denominator) and scale into fp16 OTn.
                for t in range(2):
                    psb = psp.tile([128, 1024], f32, tag="ps", name=f"psb{c}_{t}")
                    for p0 in range(2):
                        h = 2 * t + p0
                        mm(
                            psb[:, p0 * 512 : (p0 + 1) * 512],
                            onesr[:],
                            zr[:, h, :],
                            start=True,
                            stop=True,
                        )
                    bcr = bcp.tile([64, 2, 512], f32, tag="bcr", name=f"bcr{c}_{t}")
                    nc.vector.reciprocal_approx_fast(
                        bcr[:], psb[0:64, :].rearrange("p (a x) -> p a x", x=512)
                    )
                    for p0 in range(2):
                        h = 2 * t + p0
                        nc.vector.tensor_mul(
                            OTn[t][p0 * 64 : p0 * 64 + 64, csl],
                            otr[0:64, h, :],
                            bcr[:, p0, :],
                        )

            def out_proj(c):
                for st in range(4 * c, 4 * c + 4):
                    pso = psp.tile([128, 1024], f32, tag="ps")
                    for n in range(2):
                        for k2 in range(2):
                            mm(
                                pso[:, n * 512 : (n + 1) * 512],
                                OTn[k2][:, st * 128 : (st + 1) * 128],
                                wo_sb[:, k2, n * 512 : (n + 1) * 512],
                                start=(k2 == 0),
                                stop=(k2 == 1),
                            )
                    osb = osp.tile([128, D], mmdt, tag="osb")
                    if st % 2 == 0:
                        nc.vector.tensor_copy(osb[:], pso[:])
                    else:
                        nc.scalar.copy(osb[:], pso[:])
                    nc.sync.dma_start(outd[st * 128 : (st + 1) * 128, :], osb[:])

            # The normalize chain (DMA+DVE) for chunk c is emitted right after
            # chunk c's attention so it executes while the PE runs the next
            # chunk; the out-projection matmuls for chunk c are emitted one
            # chunk LATER so the in-order PE queue never head-of-line blocks
            # waiting for the normalize results.
            pending_proj = None
            for c in range(NSC):
                csl = slice(c * SC, (c + 1) * SC)
                proj_chunk(c)
                po = [
                    pop.tile([65, 512], f32, tag="po", name=f"po{c}_{h}")
                    for h in range(HL)
                ]
                for j in range(4 * c + 4):
                    attn_block(c, j, po)
                # po is copied to SBUF right away so its PSUM banks free for
                # the next chunk.
                otr = orp.tile([65, HL, 512], f32, tag="otr", name=f"otr{c}")
                for h in range(HL):
                    nc.vector.tensor_copy(otr[:, h, :], po[h][:])
                # fp16 row sums into row 0 of the zero-padded broadcast src
                # (gpsimd: idle engine, SBUF->SBUF) so it's ready by the time
                # the PE drains the previous chunk's out-projection below.
                nc.gpsimd.tensor_copy(zr[0:1, :, :], otr[64:65, :, :])
                if pending_proj is not None:
                    pending_proj()
                normalize(c, otr, csl)
                pending_proj = (lambda c=c: out_proj(c))
            pending_proj()

    nc.compile()
    return nc


def _get_nc():
    key = ("nc", MM_DTYPE)
    if key not in _CACHE:
        _CACHE[key] = _build()
    return _CACHE[key]


def make_in_maps(q, k, v, Wq, bq, Wk, bk, Wv, bv, Wo, bo):
    """Host-side shard prep: per-core input dict."""
    f32 = np.float32
    md = {"f16": np.float16, "f32r": f32, "f32": f32}[MM_DTYPE]
    masks = (
        np.arange(SC)[None, None, :]
        >= (128 * np.arange(4)[None, :, None] + np.arange(128)[:, None, None])
    ).astype(md)
    # per-batch transposes shared by the 4 cores of each batch
    xqT = [np.ascontiguousarray(q[b].T.astype(md)) for b in range(2)]
    xkT = [np.ascontiguousarray(k[b].T.astype(md)) for b in range(2)]
    xvT = [np.ascontiguousarray(v[b].T.astype(md)) for b in range(2)]
    in_maps = []
    for c in range(8):
        b, g = c // 4, c % 4
        sl = slice(DL * g, DL * (g + 1))
        in_maps.append(
            {
                "xqT": xqT[b],
                "xkT": xkT[b],
                "xvT": xvT[b],
                "wqT": np.ascontiguousarray(((Wq[sl, :].T) * f32(0.125)).astype(md)),
                "wkT": np.ascontiguousarray(Wk[sl, :].T.astype(md)),
                "wvT": np.ascontiguousarray(Wv[sl, :].T.astype(md)),
                "woT": np.ascontiguousarray(Wo[:, sl].T.astype(md)),
                "bqd": np.ascontiguousarray((bq[sl] * f32(0.125)).reshape(2, 128).T),
                "bkd": np.ascontiguousarray(bk[sl].reshape(2, 128).T),
                "maskd": masks,
            }
        )
    return in_maps


def kernel(q, k, v, Wq, bq, Wk, bk, Wv, bv, Wo, bo):
    from concourse.bass_utils import run_bass_kernel_spmd

    args = [np.asarray(a, dtype=np.float32) for a in (q, k, v, Wq, bq, Wk, bk, Wv, bv, Wo, bo)]
    q, k, v, Wq, bq, Wk, bk, Wv, bv, Wo, bo = args
    nc = _get_nc()
    in_maps = make_in_maps(q, k, v, Wq, bq, Wk, bk, Wv, bv, Wo, bo)
    tmpdir = os.environ.get("BASS_KERNEL_TMPDIR") or None
    res = run_bass_kernel_spmd(nc, in_maps, list(range(8)), trace=TRACE, tmpdir=tmpdir)
    if TRACE and res.exec_time_ns is not None:
        print(f"HW exec time: {res.exec_time_ns} ns")
        print(f"HW exec time mean: {res.mean_exec_time_ns} ns")
    out = np.zeros((2, S, D), np.float32)
    for c in range(8):
        out[c // 4] += res.results[c]["out"]
    out += (bv @ Wo.T + bo)[None, None, :]
    return out


# revision 19
# speedup vs baseline: 1.2804x; 1.1229x over previous
"""Multi-head causal self-attention (B=2, S=2048, D=1024, H=16) on 8 TRN2 cores.

Sharding: core c handles batch b = c//4 and head group g = c%4 (4 heads,
256 output dims). W_q/W_k/W_v are split column-wise per head group, W_o
row-wise; each core computes a partial [S, D] output product which the host
sums per batch (plus the (bv @ Wo.T + bo) row, exact because softmax rows
sum to 1).

Device kernel per core, fully interleaved per 512-query chunk so the PE,
ACT and DVE engines all stay busy across the whole timeline (a separate
projection phase would leave ACT idle for its first ~40us and the exp
stream would then dominate the attention phase):
  per chunk c:
    QT/KT[dl, s-chunk] = w.T @ xT (+ bias)   PE full-array, DVE bias-add
    V[s-chunk, dl|1]   = xT.T @ wv            (ones col -> row sums ride PV)
    scoresT[sk, sq] = KT_h.T @ QT_h   2 heads at partitions 0-63/64-127,
        64x128 PE row tiling -> the pair runs concurrently
    PT = exp(scoresT) (ACT) * causal mask (GPSIMD)
    po_h[dv|sum, sq] += [V_h|1].T @ PT        N=512-x0 streams
  epilogue (deferred one chunk so it hides under the next attention loop):
    rr fp16 = cast(po sums row)  -> row 0 of a zeroed [128,...] tile
    psb = onesr.T @ rr            full-array broadcast (rows 1-127 zero)
    bcr = 1/psb                   DVE reciprocal, OTn = po * bcr (fp16)
    out[s, :] = OTn.T-slice @ woT (fp16 partial, summed on host)
"""

import os
import sys

import numpy as np

# concourse (Bass/Tile) normally comes from PYTHONPATH; fall back to the
# container's copy when run from a bare directory.
for _p in ("/root/.axon_site/_ro/trn_rl_repo", "/opt/trn_rl_repo"):
    if _p not in sys.path and os.path.isdir(_p):
        sys.path.append(_p)

S = 2048
D = 1024
HL = 4          # heads per core
DL = 256        # local head dims per core
SC = 512        # sq chunk width
NSC = S // SC   # 4 chunks
NKT = S // 128  # 16 sk tiles
KC = D // 128   # 8 contraction chunks for the projections

# Matmul operand dtype: fp16 streams 1 col/cycle on the PE (fp32r: 2, fp32: 4)
# and halves the x/w DMA. fp16 is safe here: max exp(score) ~ 490 << 65504,
# verified rel err ~7e-4 end to end.
MM_DTYPE = os.environ.get("BASS_MM_DTYPE", "f16")
TRACE = os.environ.get("BASS_KERNEL_TRACE", "0") == "1"

_CACHE = {}


def _build():
    import concourse.bass as bass
    import concourse.mybir as mybir
    import concourse.tile as tile
    from concourse import bacc

    dt = mybir.dt
    f32 = dt.float32
    mmdt = {"f16": dt.float16, "f32r": dt.float32r, "f32": dt.float32}[MM_DTYPE]

    nc = bacc.Bacc("TRN2", target_bir_lowering=False, debug=False)

    xqT = nc.dram_tensor("xqT", [D, S], mmdt, kind="ExternalInput").ap()
    xkT = nc.dram_tensor("xkT", [D, S], mmdt, kind="ExternalInput").ap()
    xvT = nc.dram_tensor("xvT", [D, S], mmdt, kind="ExternalInput").ap()
    wqT = nc.dram_tensor("wqT", [D, DL], mmdt, kind="ExternalInput").ap()
    wkT = nc.dram_tensor("wkT", [D, DL], mmdt, kind="ExternalInput").ap()
    wvT = nc.dram_tensor("wvT", [D, DL], mmdt, kind="ExternalInput").ap()
    woT = nc.dram_tensor("woT", [DL, D], mmdt, kind="ExternalInput").ap()
    bqd = nc.dram_tensor("bqd", [128, 2], f32, kind="ExternalInput").ap()
    bkd = nc.dram_tensor("bkd", [128, 2], f32, kind="ExternalInput").ap()
    maskd = nc.dram_tensor("maskd", [128, 4, SC], mmdt, kind="ExternalInput").ap()
    outd = nc.dram_tensor("out", [S, D], mmdt, kind="ExternalOutput").ap()

    Exp = mybir.ActivationFunctionType.Exp

    def mm(ps, lhsT, rhs, start, stop):
        nc.tensor.matmul(ps, lhsT, rhs, start=start, stop=stop)

    with tile.TileContext(nc) as tc:
        with (
            tc.tile_pool(name="const", bufs=1) as constp,
            tc.tile_pool(name="wq", bufs=1) as wqp,
            tc.tile_pool(name="wk", bufs=1) as wkp,
            tc.tile_pool(name="wv", bufs=1) as wvp,
            tc.tile_pool(name="wo", bufs=1) as wop,
            tc.tile_pool(name="x", bufs=6) as xp,
            tc.tile_pool(name="qt", bufs=2) as qtp,
            tc.tile_pool(name="kt", bufs=2) as ktp,
            tc.tile_pool(name="v", bufs=NKT) as vp,
            tc.tile_pool(name="pt", bufs=4) as ptp,
            tc.tile_pool(name="otn", bufs=2) as otp,
            tc.tile_pool(name="otr", bufs=2) as orp,
            tc.tile_pool(name="bcr", bufs=2) as bcp,
            tc.tile_pool(name="osb", bufs=3) as osp,
            tc.tile_pool(name="ps", bufs=2, space="PSUM") as psp,
            tc.tile_pool(name="po", bufs=4, space="PSUM") as pop,
        ):
            ones_f32 = constp.tile([128, 64], f32, tag="ones_f32")
            nc.vector.memset(ones_f32[:], 1.0)
            ones16 = constp.tile([128, 64], mmdt, tag="ones16")
            nc.vector.tensor_copy(ones16[:], ones_f32[:])
            bq_sb = constp.tile([128, 2], f32, tag="bq")
            nc.sync.dma_start(bq_sb[:], bqd[:])
            bk_sb = constp.tile([128, 2], f32, tag="bk")
            nc.sync.dma_start(bk_sb[:], bkd[:])
            mask_sb = constp.tile([128, 4, SC], mmdt, tag="mask")
            nc.sync.dma_start(mask_sb[:], maskd[:])

            wq_sb = wqp.tile([128, KC, DL], mmdt, tag="wq")
            nc.sync.dma_start(wq_sb[:], wqT.rearrange("(kc p) n -> p kc n", p=128))
            wk_sb = wkp.tile([128, KC, DL], mmdt, tag="wk")
            nc.sync.dma_start(wk_sb[:], wkT.rearrange("(kc p) n -> p kc n", p=128))
            wv_sb = wvp.tile([128, KC, DL], mmdt, tag="wv")
            nc.sync.dma_start(wv_sb[:], wvT.rearrange("(kc p) n -> p kc n", p=128))
            wo_sb = wop.tile([128, 2, D], mmdt, tag="wo")
            nc.sync.dma_start(wo_sb[:], woT.rearrange("(kc p) n -> p kc n", p=128))

            QT = [qtp.tile([128, S], mmdt, tag="qt", name=f"qt{i}") for i in range(2)]
            KT = [ktp.tile([128, S], mmdt, tag="kt", name=f"kt{i}") for i in range(2)]
            OTn = [otp.tile([128, S], mmdt, tag="otn", name=f"otn{i}") for i in range(2)]
            Vt = [vp.tile([128, HL * 65], mmdt, tag="v", name=f"v{i}") for i in range(NKT)]

            xqr = xqT.rearrange("(kc p) s -> p kc s", p=128)
            xkr = xkT.rearrange("(kc p) s -> p kc s", p=128)
            xvr = xvT.rearrange("(kc p) s -> p kc s", p=128)

            def proj_chunk(sc):
                ssl = slice(sc * SC, (sc + 1) * SC)
                # QT / KT
                for xr, w_sb, dstT, b_sb in (
                    (xqr, wq_sb, QT, bq_sb),
                    (xkr, wk_sb, KT, bk_sb),
                ):
                    xt = xp.tile([128, KC, SC], mmdt, tag="x")
                    nc.sync.dma_start(xt[:], xr[:, :, ssl])
                    ps = psp.tile([128, 1024], f32, tag="ps")
                    for t in range(2):
                        for kc in range(KC):
                            mm(
                                ps[:, t * 512 : (t + 1) * 512],
                                w_sb[:, kc, t * 128 : (t + 1) * 128],
                                xt[:, kc, :],
                                start=(kc == 0),
                                stop=(kc == KC - 1),
                            )
                    for t in range(2):
                        nc.vector.tensor_add(
                            dstT[t][:, ssl],
                            ps[:, t * 512 : (t + 1) * 512],
                            b_sb[:, t : t + 1].broadcast_to([128, SC]),
                        )
                # V
                xt = xp.tile([128, KC, SC], mmdt, tag="x")
                nc.sync.dma_start(xt[:], xvr[:, :, ssl])
                for pair in range(2):
                    ps = psp.tile([128, 1024], f32, tag="ps")
                    for sub in range(2):
                        st = sc * 4 + pair * 2 + sub
                        off = sub * 512
                        for kc in range(KC):
                            mm(
                                ps[:, off : off + DL],
                                xt[:, kc, (pair * 2 + sub) * 128 : (pair * 2 + sub + 1) * 128],
                                wv_sb[:, kc, :],
                                start=(kc == 0),
                                stop=(kc == KC - 1),
                            )
                        dst = Vt[st].rearrange("p (h x) -> p h x", x=65)
                        nc.vector.tensor_copy(
                            dst[:, :, 0:64],
                            ps[:, off : off + DL].rearrange("p (h x) -> p h x", x=64),
                        )
                        nc.vector.tensor_copy(
                            dst[:, :, 64:65],
                            ones_f32[:, None, 0:1].broadcast_to([128, HL, 1]),
                        )

            def attn_block(c, j, po):
                d = j - 4 * c  # >= 0 on the block diagonal
                x0 = max(0, 128 * d)  # first non-fully-masked sq column
                jmax = 4 * c + 3
                # all 4 score matmuls (64x128 row-tiled mode), then all 4 PV
                # matmuls (full-array mode): 2 PE tiling-mode switches per j
                # instead of 4 (each switch drains the systolic array)
                pts = []
                for pr in range(2):
                    ps = psp.tile([128, 1024], f32, tag="ps")
                    for h2 in range(2):
                        h = pr * 2 + h2
                        t, p0 = divmod(h, 2)
                        psl = slice(p0 * 64, p0 * 64 + 64)
                        mm(
                            ps[:, h2 * 512 + x0 : (h2 + 1) * 512],
                            KT[t][psl, j * 128 : (j + 1) * 128],
                            QT[t][psl, c * SC + x0 : (c + 1) * SC],
                            start=True,
                            stop=True,
                        )
                    pt = ptp.tile([128, 1024], mmdt, tag="pt")
                    psv = ps.rearrange("p (h x) -> p h x", x=512)
                    ptv = pt.rearrange("p (h x) -> p h x", x=512)
                    nc.scalar.activation(ptv[:, :, x0:], psv[:, :, x0:], Exp)
                    if d >= 0:
                        # triangular mask on the 128-wide diagonal block
                        nc.gpsimd.tensor_mul(
                            ptv[:, :, x0 : x0 + 128],
                            ptv[:, :, x0 : x0 + 128],
                            mask_sb[:, 0:1, 0:128].broadcast_to([128, 2, 128]),
                        )
                    pts.append(pt)
                for pr in range(2):
                    for h2 in range(2):
                        h = pr * 2 + h2
                        mm(
                            po[h][:, x0:],
                            Vt[j][:, 65 * h : 65 * h + 65],
                            pts[pr][:, h2 * 512 + x0 : (h2 + 1) * 512],
                            start=(j == 0),
                            stop=(j == jmax),
                        )

            def normalize(c, otr, csl):
                # Broadcast the fp16 row sums across 64 partitions with a
                # 1-partition outer-product matmul straight from otr, then
                # 1/x with the fast-approx custom DVE op (18-bit, plenty for
                # a softmax denominator) and scale into fp16 OTn.
                for t in range(2):
                    psb = psp.tile([128, 1024], f32, tag="ps", name=f"psb{c}_{t}")
                    for p0 in range(2):
                        h = 2 * t + p0
                        # lhsT row 64 matches the rhs base partition (the
                        # sums row) -> tile_position (64, 0), a 32x64 tile
                        mm(
                            psb[0:64, p0 * 512 : (p0 + 1) * 512],
                            ones16[64:65, :],
                            otr[64:65, h, :],
                            start=True,
                            stop=True,
                        )
                    bcr = bcp.tile([64, 2, 512], f32, tag="bcr", name=f"bcr{c}_{t}")
                    nc.vector.reciprocal_approx_fast(
                        bcr[:], psb[0:64, :].rearrange("p (a x) -> p a x", x=512)
                    )
                    for p0 in range(2):
                        h = 2 * t + p0
                        nc.vector.tensor_mul(
                            OTn[t][p0 * 64 : p0 * 64 + 64, csl],
                            otr[0:64, h, :],
                            bcr[:, p0, :],
                        )

            def out_proj(c):
                for st in range(4 * c, 4 * c + 4):
                    pso = psp.tile([128, 1024], f32, tag="ps")
                    for n in range(2):
                        for k2 in range(2):
                            mm(
                                pso[:, n * 512 : (n + 1) * 512],
                                OTn[k2][:, st * 128 : (st + 1) * 128],
                                wo_sb[:, k2, n * 512 : (n + 1) * 512],
                                start=(k2 == 0),
                                stop=(k2 == 1),
                            )
                    osb = osp.tile([128, D], mmdt, tag="osb")
                    if st % 2 == 0:
                        nc.vector.tensor_copy(osb[:], pso[:])
                    else:
                        nc.scalar.copy(osb[:], pso[:])
                    nc.sync.dma_start(outd[st * 128 : (st + 1) * 128, :], osb[:])

            # The normalize chain (DMA+DVE) for chunk c is emitted right after
            # chunk c's attention so it executes while the PE runs the next
            # chunk; the out-projection matmuls for chunk c are emitted one
            # chunk LATER so the in-order PE queue never head-of-line blocks
            # waiting for the normalize results.
            pending_proj = None
            for c in range(NSC):
                csl = slice(c * SC, (c + 1) * SC)
                proj_chunk(c)
                po = [
                    pop.tile([65, 512], f32, tag="po", name=f"po{c}_{h}")
                    for h in range(HL)
                ]
                for j in range(4 * c + 4):
                    attn_block(c, j, po)
                # po is copied to SBUF right away so its PSUM banks free for
                # the next chunk.
                # fp16 staging: po values are O(1..2e3), fp16's 0.05% step is
                # well inside the output tolerance and halves the copy traffic
                otr = orp.tile([65, HL, 512], mmdt, tag="otr", name=f"otr{c}")
                for h in range(HL):
                    nc.vector.tensor_copy(otr[:, h, :], po[h][:])
                if pending_proj is not None:
                    pending_proj()
                normalize(c, otr, csl)
                pending_proj = (lambda c=c: out_proj(c))
            pending_proj()

    nc.compile()
    return nc


def _get_nc():
    key = ("nc", MM_DTYPE)
    if key not in _CACHE:
        _CACHE[key] = _build()
    return _CACHE[key]


def make_in_maps(q, k, v, Wq, bq, Wk, bk, Wv, bv, Wo, bo):
    """Host-side shard prep: per-core input dict."""
    f32 = np.float32
    md = {"f16": np.float16, "f32r": f32, "f32": f32}[MM_DTYPE]
    masks = (
        np.arange(SC)[None, None, :]
        >= (128 * np.arange(4)[None, :, None] + np.arange(128)[:, None, None])
    ).astype(md)
    # per-batch transposes shared by the 4 cores of each batch
    xqT = [np.ascontiguousarray(q[b].T.astype(md)) for b in range(2)]
    xkT = [np.ascontiguousarray(k[b].T.astype(md)) for b in range(2)]
    xvT = [np.ascontiguousarray(v[b].T.astype(md)) for b in range(2)]
    in_maps = []
    for c in range(8):
        b, g = c // 4, c % 4
        sl = slice(DL * g, DL * (g + 1))
        in_maps.append(
            {
                "xqT": xqT[b],
                "xkT": xkT[b],
                "xvT": xvT[b],
                "wqT": np.ascontiguousarray(((Wq[sl, :].T) * f32(0.125)).astype(md)),
                "wkT": np.ascontiguousarray(Wk[sl, :].T.astype(md)),
                "wvT": np.ascontiguousarray(Wv[sl, :].T.astype(md)),
                "woT": np.ascontiguousarray(Wo[:, sl].T.astype(md)),
                "bqd": np.ascontiguousarray((bq[sl] * f32(0.125)).reshape(2, 128).T),
                "bkd": np.ascontiguousarray(bk[sl].reshape(2, 128).T),
                "maskd": masks,
            }
        )
    return in_maps


def kernel(q, k, v, Wq, bq, Wk, bk, Wv, bv, Wo, bo):
    from concourse.bass_utils import run_bass_kernel_spmd

    args = [np.asarray(a, dtype=np.float32) for a in (q, k, v, Wq, bq, Wk, bk, Wv, bv, Wo, bo)]
    q, k, v, Wq, bq, Wk, bk, Wv, bv, Wo, bo = args
    nc = _get_nc()
    in_maps = make_in_maps(q, k, v, Wq, bq, Wk, bk, Wv, bv, Wo, bo)
    tmpdir = os.environ.get("BASS_KERNEL_TMPDIR") or None
    res = run_bass_kernel_spmd(nc, in_maps, list(range(8)), trace=TRACE, tmpdir=tmpdir)
    if TRACE and res.exec_time_ns is not None:
        print(f"HW exec time: {res.exec_time_ns} ns")
        print(f"HW exec time mean: {res.mean_exec_time_ns} ns")
    out = np.zeros((2, S, D), np.float32)
    for c in range(8):
        out[c // 4] += res.results[c]["out"]
    out += (bv @ Wo.T + bo)[None, None, :]
    return out
